# revision 59
# baseline (speedup 1.0000x reference)
"""Trainium2 Bass kernel for nn_Attention (softmax over the QUERY axis).

reference:
    scores  = einsum("bqd,bkd->bqk", query, context)           # [B,Q,K]
    scores  = where(mask[:, :, None] == 0, -inf, scores)       # mask whole q rows
    attw    = softmax(scores, axis=1)                          # softmax over q !
    mix     = einsum("bqk,bkd->bqd", attw, context)            # [B,Q,D]
    out     = tanh(concat([mix, query], -1) @ w_out.T)         # [B,Q,D]
    returns (out, attw)

Strategy: pure data parallel over B (16 batches / 8 cores = 2 per core).

On-chip dataflow (per batch). TensorE matmul computes lhsT.T @ rhs with the
contraction on partitions, so everything is kept transposed:
    scoresT[k,q] = cT.T @ qT          (cT,qT built with PE transposes)
    softmax along the free (q) axis of scoresT
    mixT[d,q]    = ctx.T @ pT         (ctx in native layout)
    out[q,d]     = combinedT.T @ w_outT,  combinedT = [mixT; qT]
attw needs [q,k] layout, so p tiles are PE-transposed back before the DMA out.

Precision: the scores matmul feeds exp(), so it runs in float32r (the PE's
reduced fp32 mode: 1 cycle/row at free-dim >= 256, ~12-13 mantissa bits,
measured ~1.5e-4 per-matmul error) — score_passes=2, the default.
Alternatives kept behind the flag: 3 = fp16 hi/lo 3-pass (near-exact scores,
~25% slower), 1 = plain fp16 (faster, ~2x the error).  Downstream
(mix / out) runs single-pass fp16.  Measured vs the fp32 reference:
out 6.7e-4 norm-rel / 1.7e-2 absmax-of-scale, attw 8.3e-4 / 5.7e-3.

SBUF is tight (~208KB/partition): only qT hi/lo, the mask bias row and small
work tiles stay resident; pT, ctx(f16) and w_outT live in DRAM scratch and
are streamed back in the mix/out phases.
"""

import os
import sys

import numpy as np

sys.path.insert(0, "/opt/trn_rl_repo")

P = 128
MMF = 512  # matmul free dim (one fp32 PSUM bank)
KG = 4  # k-tiles per attw transpose/DMA group

_NC_CACHE = {}


def build_nc(B_pc=2, Q=2048, K=2048, D=1024, score_passes=3, qa=None):
    """Build the per-core Bass program (SPMD; every core runs the same NEFF).

    qa: active-query extent.  The host permutes each batch's rows so the
    unmasked (attention_mask==1) rows come first; rows [0, qa) get the full
    scores/softmax/attw/mix treatment, rows [qa, Q) are masked rows whose
    attention weights are exactly zero, so only the query-half of the output
    projection runs for them.  qa=Q disables the optimization.
    """
    import concourse.bass as bass
    import concourse.mybir as mybir
    import concourse.tile as tile
    from concourse import bacc
    from concourse.masks import make_identity
    from contextlib import ExitStack

    f32 = mybir.dt.float32
    f16 = mybir.dt.float16
    i32 = mybir.dt.int32
    Alu = mybir.AluOpType
    Act = mybir.ActivationFunctionType

    E = 2 * D
    if qa is None:
        qa = Q
    KT = K // P          # k tiles
    QT = Q // P          # q tiles (full)
    QTA = qa // P        # active q tiles
    DC = D // P          # d chunks
    EC = E // P          # e chunks
    QF = next(c for c in (512, 384, 256, 128) if qa % c == 0)  # scores/mix free
    QC = qa // QF        # active q chunks / stripes
    OF = min(MMF, D)     # out free size
    OC = D // OF
    NG = KT // KG        # attw groups
    assert qa % P == 0 and 0 < qa <= Q and Q % P == 0
    assert K % (P * KG) == 0 and D % P == 0

    nc = bacc.Bacc("TRN2", target_bir_lowering=False, debug=False, num_devices=8)

    query = nc.dram_tensor("query", [B_pc, Q, D], f32, kind="ExternalInput").ap()
    context = nc.dram_tensor("context", [B_pc, K, D], f32, kind="ExternalInput").ap()
    amask = nc.dram_tensor("attention_mask", [B_pc, Q], i32, kind="ExternalInput").ap()
    w_out = nc.dram_tensor("w_out", [D, E], f32, kind="ExternalInput").ap()
    out = nc.dram_tensor("out", [B_pc, Q, D], f32, kind="ExternalOutput").ap()
    attw = nc.dram_tensor("attw", [B_pc, Q, K], f32, kind="ExternalOutput").ap()

    with tile.TileContext(nc) as tc, ExitStack() as ctx:
        ps_tp = ctx.enter_context(tc.tile_pool(name="ps_tp", bufs=2, space="PSUM"))
        ps_mm = ctx.enter_context(tc.tile_pool(name="ps_mm", bufs=4, space="PSUM"))
        ps_pt = ctx.enter_context(tc.tile_pool(name="ps_pt", bufs=2, space="PSUM"))

        const = ctx.enter_context(tc.tile_pool(name="const", bufs=1))
        dram = ctx.enter_context(tc.tile_pool(name="dram", bufs=2, space="DRAM"))

        ident32 = const.tile([P, P], f32)
        make_identity(nc, ident32)
        ident16 = const.tile([P, P], f16)
        make_identity(nc, ident16)
        f32r = mybir.dt.float32r
        use_f32r = score_passes == 2
        if use_f32r:
            ones_f = const.tile([1, P], f32)
            nc.gpsimd.memset(ones_f[:], 1.0)
            ones16 = const.tile([1, P], f32r)
            nc.vector.tensor_copy(ones16[:], ones_f[:])
        else:
            ones16 = const.tile([1, P], f16)
            nc.gpsimd.memset(ones16[:], 1.0)

        # ---- w_outT[e, d] -> DRAM scratch [OC, P, EC, OF] f16 ----
        # Layout chosen so the out-phase readback is one contiguous
        # EC*OF*2-byte run per partition (single-descriptor DMA).
        w_dram = dram.tile([OC, P, EC, OF], f16)
        with tc.tile_pool(name="wnat", bufs=2) as wnat, \
             tc.tile_pool(name="wsb", bufs=2) as wsb:
            for dt_ in range(DC):
                dc2, od0 = (dt_ * P) // OF, (dt_ * P) % OF
                wtile = wnat.tile([P, E], f32, tag="wnat")
                nc.sync.dma_start(wtile[:], w_out[dt_ * P:(dt_ + 1) * P, :])
                for j0 in range(0, EC, 4):
                    nj = min(4, EC - j0)
                    ps = ps_tp.tile([P, 4 * P], f32, tag="tp")
                    for a in range(nj):
                        nc.tensor.transpose(
                            ps[:, a * P:(a + 1) * P],
                            wtile[:, (j0 + a) * P:(j0 + a + 1) * P],
                            ident32,
                        )
                    wsl = wsb.tile([P, 4 * P], f16, tag="wsb")
                    nc.vector.tensor_copy(wsl[:, :nj * P], ps[:, :nj * P])
                    nc.sync.dma_start(
                        w_dram[dc2, :, j0:j0 + nj, od0:od0 + P],
                        wsl[:, :nj * P].rearrange("p (j d) -> p j d", j=nj),
                    )

        # ---- persistent per-batch pools ----
        perb = ctx.enter_context(tc.tile_pool(name="perb", bufs=1))
        mrow = ctx.enter_context(tc.tile_pool(name="mrow", bufs=1))
        nat = ctx.enter_context(tc.tile_pool(name="nat", bufs=3))
        chh = ctx.enter_context(tc.tile_pool(name="chh", bufs=2))
        ctp = ctx.enter_context(tc.tile_pool(name="ctp", bufs=2))
        stp = ctx.enter_context(tc.tile_pool(name="stp", bufs=1))
        ptp = ctx.enter_context(tc.tile_pool(name="ptp", bufs=KG + 1))
        smal = ctx.enter_context(tc.tile_pool(name="smal", bufs=4))
        stg = ctx.enter_context(tc.tile_pool(name="stg", bufs=2))
        mixp = ctx.enter_context(tc.tile_pool(name="mixp", bufs=1))
        strm = ctx.enter_context(tc.tile_pool(name="strm", bufs=2))
        cstr = ctx.enter_context(tc.tile_pool(name="cstr", bufs=2))
        outp = ctx.enter_context(tc.tile_pool(name="outp", bufs=3))
        qstg = ctx.enter_context(tc.tile_pool(name="qstg", bufs=2)) if use_f32r else None

        # pT spill: [b, stripe, partition, kt, qf] -> stripe readback is one
        # contiguous KT*QF*2-byte run per partition.
        pT_dram = dram.tile([B_pc, QC, P, KT, QF], f16)

        for b in range(B_pc):
            # ctx(f16) spill: [j, partition(k), kt, pd] -> mix readback of
            # [j] is one contiguous KT*P*2-byte run per partition.
            ctx_dram = dram.tile([DC, P, KT, P], f16, tag="ctxd", name="ctxd")

            # ---- mask bias row: (m - 1) * 60000, f16 (folded into the scores
            # matmul as a rank-1 ones x mb pass; exp(-60000+x) == 0 in f32) ----
            mrow_i = mrow.tile([1, Q], i32, tag="mrow_i")
            nc.sync.dma_start(mrow_i[:], amask[b, None, :])
            mrow_f = mrow.tile([1, Q], f32, tag="mrow_f")
            nc.vector.tensor_copy(mrow_f[:], mrow_i[:])
            mb16 = mrow.tile([1, Q], f32r if use_f32r else f16, tag="mb16")
            nc.vector.tensor_scalar(
                mb16[:], mrow_f[:], 60000.0, -60000.0, op0=Alu.mult, op1=Alu.add
            )

            # ---- qT (hi/lo fp16, or f32 for the f32r mode) ----
            # layout [P, DC, Q], partition = d in chunk
            qT_hi = perb.tile([P, DC, Q], f32r if use_f32r else f16, tag="qt_hi")
            qT_lo = (
                perb.tile([P, DC, Q], f16, tag="qt_lo", name="qt_lo")
                if score_passes == 3
                else None
            )
            for t in range(QT):
                qtile = nat.tile([P, D], f32, tag="nat")
                nc.sync.dma_start(qtile[:], query[b, t * P:(t + 1) * P, :])
                for j0 in range(0, DC, 4):
                    nj = min(4, DC - j0)
                    ps = ps_tp.tile([P, 4 * P], f32, tag="tp")
                    for a in range(nj):
                        nc.tensor.transpose(
                            ps[:, a * P:(a + 1) * P],
                            qtile[:, (j0 + a) * P:(j0 + a + 1) * P],
                            ident32,
                        )
                    psv = ps[:, :nj * P].rearrange("p (a c) -> p a c", a=nj)
                    dst = qT_hi[:, j0:j0 + nj, t * P:(t + 1) * P]
                    if t % 2 == 0 or use_f32r:
                        nc.vector.tensor_copy(dst, psv)
                    else:
                        nc.scalar.copy(dst, psv)
                    if qT_lo is not None:
                        nc.vector.tensor_tensor(
                            qT_lo[:, j0:j0 + nj, t * P:(t + 1) * P],
                            psv,
                            dst,
                            Alu.subtract,
                        )

            # ---- k loop: scores -> softmax -> pT (spilled to DRAM) ----
            pT_tiles = [None] * KT

            def emit_attw_group(g, b=b):
                # transpose pT tiles [kG..] back to [q, k] layout and DMA out
                for qb in range(QTA):
                    pp = ps_pt.tile([P, KG * P], f16, tag="pt")
                    for a in range(KG):
                        nc.tensor.transpose(
                            pp[:, a * P:(a + 1) * P],
                            pT_tiles[g * KG + a][:, qb * P:(qb + 1) * P],
                            ident16,
                        )
                    st = stg.tile([P, KG * P], f32, tag="stg")
                    if qb % 2 == 0:
                        nc.vector.tensor_copy(st[:], pp[:])
                    else:
                        nc.scalar.copy(st[:], pp[:])
                    nc.sync.dma_start(
                        attw[b, qb * P:(qb + 1) * P, g * KG * P:(g + 1) * KG * P],
                        st[:],
                    )

            for kt in range(KT):
                ctile = nat.tile([P, D], f32, tag="nat")
                nc.sync.dma_start(ctile[:], context[b, kt * P:(kt + 1) * P, :])
                chf = chh.tile([P, D], f16, tag="chh")
                nc.gpsimd.tensor_copy(chf[:], ctile[:])
                nc.sync.dma_start(
                    ctx_dram[:, :, kt, :].rearrange("j pk pd -> pk j pd"),
                    chf[:].rearrange("pk (j pd) -> pk j pd", j=DC),
                )

                cT_hi = ctp.tile([P, DC, P], f32r if use_f32r else f16, tag="ct_hi")
                cT_lo = (
                    ctp.tile([P, DC, P], f16, tag="ct_lo", name="ct_lo")
                    if score_passes == 3
                    else None
                )
                for j0 in range(0, DC, 4):
                    nj = min(4, DC - j0)
                    ps = ps_tp.tile([P, 4 * P], f32, tag="tp")
                    for a in range(nj):
                        nc.tensor.transpose(
                            ps[:, a * P:(a + 1) * P],
                            ctile[:, (j0 + a) * P:(j0 + a + 1) * P],
                            ident32,
                        )
                    psv = ps[:, :nj * P].rearrange("p (a c) -> p a c", a=nj)
                    dst = cT_hi[:, j0:j0 + nj, :]
                    if kt % 2 == 0 or use_f32r:
                        nc.vector.tensor_copy(dst, psv)
                    else:
                        nc.scalar.copy(dst, psv)
                    if cT_lo is not None:
                        nc.vector.tensor_tensor(
                            cT_lo[:, j0:j0 + nj, :],
                            psv,
                            dst,
                            Alu.subtract,
                        )

                # scores for this k tile, all active q chunks -> sT (masked, f32)
                sT = stp.tile([P, qa], f32, tag="sT")
                if score_passes == 3:
                    passes = [(cT_hi, qT_hi), (cT_hi, qT_lo), (cT_lo, qT_hi)]
                else:
                    passes = [(cT_hi, qT_hi)]
                # All QC chunk accumulators live at once so each lhsT weight
                # tile is loaded once and reused across the QC matmuls
                # (f32r weight loads are slow; fp16 FWL also benefits).
                nmm = len(passes) * DC
                pmms = [
                    ps_mm.tile([P, QF], f32, tag="mm", name=f"mm{qc}")
                    for qc in range(QC)
                ]
                for qc in range(QC):
                    # rank-1 mask pass: psum += ones[k] x maskbias[q]
                    nc.tensor.matmul(
                        pmms[qc][:],
                        lhsT=ones16[:],
                        rhs=mb16[:, qc * QF:(qc + 1) * QF],
                        start=True,
                        stop=False,
                    )
                i = 0
                for lhsP, rhsP in passes:
                    for j in range(DC):
                        for qc in range(QC):
                            nc.tensor.matmul(
                                pmms[qc][:],
                                lhsT=lhsP[:, j, :],
                                rhs=rhsP[:, j, qc * QF:(qc + 1) * QF],
                                start=False,
                                stop=(i == nmm - 1),
                            )
                        i += 1
                for qc in range(QC):
                    if qc % 2 == 0:
                        nc.scalar.copy(sT[:, qc * QF:(qc + 1) * QF], pmms[qc][:])
                    else:
                        nc.vector.tensor_copy(sT[:, qc * QF:(qc + 1) * QF], pmms[qc][:])

                # softmax over q (free axis)
                negmx = smal.tile([P, 1], f32, tag="negmx")
                nc.vector.reduce_max(
                    negmx[:], sT[:], axis=mybir.AxisListType.X, negate=True
                )
                pT = ptp.tile([P, qa], f16, tag="pT")
                denom = smal.tile([P, 1], f32, tag="denom")
                nc.scalar.activation(
                    pT[:], sT[:], Act.Exp, bias=negmx[:], scale=1.0, accum_out=denom[:]
                )
                recip = smal.tile([P, 1], f32, tag="recip")
                nc.vector.reciprocal(recip[:], denom[:])
                nc.vector.tensor_scalar_mul(pT[:], pT[:], recip[:])
                pT_tiles[kt] = pT
                nc.gpsimd.dma_start(
                    pT_dram[b, :, :, kt, :].rearrange("s p f -> p s f"),
                    pT[:].rearrange("p (s f) -> p s f", s=QC),
                )

                # emit attw transposes for the previous completed group (keeps
                # PE fed: by now that group's softmax has long finished)
                if kt % KG == 0 and kt > 0:
                    emit_attw_group(kt // KG - 1)
            emit_attw_group(NG - 1)

            # ---- mix + out, per q stripe ----
            for s in range(QC):
                pst = strm.tile([P, KT, QF], f16, tag="strm", name="pstrip")
                nc.sync.dma_start(pst[:], pT_dram[b, s])
                mixT = mixp.tile([P, DC, QF], f16, tag="mixT")
                for j in range(DC):
                    cs = cstr.tile([P, KT, P], f16, tag="cstr")
                    nc.sync.dma_start(cs[:], ctx_dram[j])
                    pmm = ps_mm.tile([P, QF], f32, tag="mm")
                    for kt in range(KT):
                        nc.tensor.matmul(
                            pmm[:],
                            lhsT=cs[:, kt, :],
                            rhs=pst[:, kt, :],
                            start=(kt == 0),
                            stop=(kt == KT - 1),
                        )
                    nc.vector.tensor_copy(mixT[:, j, :], pmm[:])

                for dc2 in range(OC):
                    ws = strm.tile([P, EC, OF], f16, tag="strm", name="wstrip")
                    nc.sync.dma_start(ws[:], w_dram[dc2])
                    for qs in range(QF // P):
                        qv = qT_hi[:, :, s * QF + qs * P:s * QF + (qs + 1) * P]
                        if use_f32r:
                            # stage the fp32 qT slice as f16 for the mixed out
                            # matmul (walrus requires matching operand dtypes)
                            qst = qstg.tile([P, DC, P], f16, tag="qstg")
                            nc.vector.tensor_copy(qst[:], qv)
                            qv = qst
                        pmm = ps_mm.tile([P, OF], f32, tag="mm")
                        for j in range(DC):
                            nc.tensor.matmul(
                                pmm[:],
                                lhsT=mixT[:, j, qs * P:(qs + 1) * P],
                                rhs=ws[:, j, :],
                                start=(j == 0),
                                stop=False,
                            )
                        for j in range(DC):
                            nc.tensor.matmul(
                                pmm[:],
                                lhsT=qv[:, j, :],
                                rhs=ws[:, DC + j, :],
                                start=False,
                                stop=(j == DC - 1),
                            )
                        osb = outp.tile([P, OF], f32, tag="osb")
                        nc.scalar.activation(osb[:], pmm[:], Act.Tanh)
                        r0 = s * QF + qs * P
                        nc.gpsimd.dma_start(
                            out[b, r0:r0 + P, dc2 * OF:(dc2 + 1) * OF], osb[:]
                        )

            # ---- masked tail rows [qa, Q): attention weights are exactly
            # zero there, so only the query half of w_out contributes ----
            for dc2 in range(OC if QTA < QT else 0):
                wst = strm.tile([P, EC, OF], f16, tag="strm", name="wtail")
                nc.sync.dma_start(wst[:], w_dram[dc2])
                for blk in range(QTA, QT):
                    qv = qT_hi[:, :, blk * P:(blk + 1) * P]
                    if use_f32r:
                        qst = qstg.tile([P, DC, P], f16, tag="qstg")
                        nc.vector.tensor_copy(qst[:], qv)
                        qv = qst
                    pmm = ps_mm.tile([P, OF], f32, tag="mm")
                    for j in range(DC):
                        nc.tensor.matmul(
                            pmm[:],
                            lhsT=qv[:, j, :],
                            rhs=wst[:, DC + j, :],
                            start=(j == 0),
                            stop=(j == DC - 1),
                        )
                    osb = outp.tile([P, OF], f32, tag="osb")
                    nc.scalar.activation(osb[:], pmm[:], Act.Tanh)
                    nc.gpsimd.dma_start(
                        out[b, blk * P:(blk + 1) * P, dc2 * OF:(dc2 + 1) * OF],
                        osb[:],
                    )

    nc.compile()
    return nc


def _get_nc(key):
    if key not in _NC_CACHE:
        _NC_CACHE[key] = build_nc(*key)
    return _NC_CACHE[key]


def run_sharded(inputs, trace=False, score_passes=2, compact=True, **spmd_kwargs):
    from concourse.bass_utils import run_bass_kernel_spmd

    query = np.ascontiguousarray(np.asarray(inputs["query"], dtype=np.float32))
    context = np.ascontiguousarray(np.asarray(inputs["context"], dtype=np.float32))
    mask = np.ascontiguousarray(np.asarray(inputs["attention_mask"], dtype=np.int32))
    w_out = np.ascontiguousarray(np.asarray(inputs["w_out"], dtype=np.float32))

    B, Q, D = query.shape
    K = context.shape[1]
    n_cores = 8
    B_pc = B // n_cores

    # Permute each batch's rows so the active (mask==1) rows come first; the
    # kernel then only runs scores/softmax/attw/mix on the first `qa` rows.
    # Pure row reordering: per-row results are numerically identical.
    qa = Q
    perms = None
    if compact:
        n_max = int(mask.sum(axis=1).max())
        qa = min(Q, max(384, -(-n_max // 384) * 384))
        if qa < Q:
            perms = np.argsort(1 - mask, axis=1, kind="stable")
            query = np.take_along_axis(query, perms[:, :, None], axis=1)
            mask = np.take_along_axis(mask, perms, axis=1)
        else:
            qa = Q

    nc = _get_nc((B_pc, Q, K, D, score_passes, qa))

    in_maps = [
        {
            "query": query[i * B_pc:(i + 1) * B_pc],
            "context": context[i * B_pc:(i + 1) * B_pc],
            "attention_mask": mask[i * B_pc:(i + 1) * B_pc],
            "w_out": w_out,
        }
        for i in range(n_cores)
    ]
    res = run_bass_kernel_spmd(
        nc, in_maps, core_ids=list(range(n_cores)), trace=trace, **spmd_kwargs
    )
    outs = res.results
    out = np.concatenate([r["out"] for r in outs], axis=0)
    attw = np.concatenate([r["attw"] for r in outs], axis=0)

    if perms is not None:
        out_f = np.empty_like(out)
        attw_f = np.zeros_like(attw)
        for b in range(B):
            out_f[b, perms[b]] = out[b]
            attw_f[b, perms[b, :qa]] = attw[b, :qa]
        out, attw = out_f, attw_f
    return out, attw, res


def kernel(**inputs):
    out, attw, _ = run_sharded(inputs)
    return out, attw


# revision 61
# speedup vs baseline: 1.1835x; 1.1835x over previous
"""Trainium2 Bass kernel for nn_Attention (softmax over the QUERY axis).

reference:
    scores  = einsum("bqd,bkd->bqk", query, context)           # [B,Q,K]
    scores  = where(mask[:, :, None] == 0, -inf, scores)       # mask whole q rows
    attw    = softmax(scores, axis=1)                          # softmax over q !
    mix     = einsum("bqk,bkd->bqd", attw, context)            # [B,Q,D]
    out     = tanh(concat([mix, query], -1) @ w_out.T)         # [B,Q,D]
    returns (out, attw)

Strategy: pure data parallel over B (16 batches / 8 cores = 2 per core).

On-chip dataflow (per batch). TensorE matmul computes lhsT.T @ rhs with the
contraction on partitions, so everything is kept transposed:
    scoresT[k,q] = cT.T @ qT          (cT,qT built with PE transposes)
    softmax along the free (q) axis of scoresT
    mixT[d,q]    = ctx.T @ pT         (ctx in native layout)
    out[q,d]     = combinedT.T @ w_outT,  combinedT = [mixT; qT]
attw needs [q,k] layout, so p tiles are PE-transposed back before the DMA out.

Precision: the scores matmul feeds exp(), so it runs in float32r (the PE's
reduced fp32 mode: 1 cycle/row at free-dim >= 256, ~12-13 mantissa bits,
measured ~1.5e-4 per-matmul error) — score_passes=2, the default.
Alternatives kept behind the flag: 3 = fp16 hi/lo 3-pass (near-exact scores,
~25% slower), 1 = plain fp16 (faster, ~2x the error).  Downstream
(mix / out) runs single-pass fp16.  Measured vs the fp32 reference:
out 6.7e-4 norm-rel / 1.7e-2 absmax-of-scale, attw 8.3e-4 / 5.7e-3.

SBUF is tight (~208KB/partition): only qT hi/lo, the mask bias row and small
work tiles stay resident; pT, ctx(f16) and w_outT live in DRAM scratch and
are streamed back in the mix/out phases.
"""

import os
import sys

import numpy as np

sys.path.insert(0, "/opt/trn_rl_repo")

P = 128
MMF = 512  # matmul free dim (one fp32 PSUM bank)
KG = 4  # k-tiles per attw transpose/DMA group

_NC_CACHE = {}


def build_nc(B_pc=2, Q=2048, K=2048, D=1024, score_passes=3, qa=None):
    """Build the per-core Bass program (SPMD; every core runs the same NEFF).

    qa: active-query extent.  The host permutes each batch's rows so the
    unmasked (attention_mask==1) rows come first; rows [0, qa) get the full
    scores/softmax/attw/mix treatment, rows [qa, Q) are masked rows whose
    attention weights are exactly zero, so only the query-half of the output
    projection runs for them.  qa=Q disables the optimization.
    """
    import concourse.bass as bass
    import concourse.mybir as mybir
    import concourse.tile as tile
    from concourse import bacc
    from concourse.masks import make_identity
    from contextlib import ExitStack

    f32 = mybir.dt.float32
    f16 = mybir.dt.float16
    i32 = mybir.dt.int32
    Alu = mybir.AluOpType
    Act = mybir.ActivationFunctionType

    E = 2 * D
    if qa is None:
        qa = Q
    KT = K // P          # k tiles
    QT = Q // P          # q tiles (full)
    QTA = qa // P        # active q tiles
    DC = D // P          # d chunks
    EC = E // P          # e chunks
    QF = next(c for c in (512, 384, 256, 128) if qa % c == 0)  # scores/mix free
    QC = qa // QF        # active q chunks / stripes
    OF = min(MMF, D)     # out free size
    OC = D // OF
    NG = KT // KG        # attw groups
    assert qa % P == 0 and 0 < qa <= Q and Q % P == 0
    assert K % (P * KG) == 0 and D % P == 0

    nc = bacc.Bacc("TRN2", target_bir_lowering=False, debug=False, num_devices=8)

    query = nc.dram_tensor("query", [B_pc, Q, D], f32, kind="ExternalInput").ap()
    context = nc.dram_tensor("context", [B_pc, K, D], f32, kind="ExternalInput").ap()
    amask = nc.dram_tensor("attention_mask", [B_pc, Q], i32, kind="ExternalInput").ap()
    w_out = nc.dram_tensor("w_out", [D, E], f32, kind="ExternalInput").ap()
    out = nc.dram_tensor("out", [B_pc, Q, D], f32, kind="ExternalOutput").ap()
    attw = nc.dram_tensor("attw", [B_pc, Q, K], f32, kind="ExternalOutput").ap()

    with tile.TileContext(nc) as tc, ExitStack() as ctx:
        ps_tp = ctx.enter_context(tc.tile_pool(name="ps_tp", bufs=2, space="PSUM"))
        ps_mm = ctx.enter_context(tc.tile_pool(name="ps_mm", bufs=4, space="PSUM"))
        ps_pt = ctx.enter_context(tc.tile_pool(name="ps_pt", bufs=2, space="PSUM"))

        const = ctx.enter_context(tc.tile_pool(name="const", bufs=1))
        dram = ctx.enter_context(tc.tile_pool(name="dram", bufs=2, space="DRAM"))

        ident32 = const.tile([P, P], f32)
        make_identity(nc, ident32)
        ident16 = const.tile([P, P], f16)
        make_identity(nc, ident16)
        f32r = mybir.dt.float32r
        use_f32r = score_passes == 2
        if use_f32r:
            ones_f = const.tile([1, P], f32)
            nc.gpsimd.memset(ones_f[:], 1.0)
            ones16 = const.tile([1, P], f32r)
            nc.vector.tensor_copy(ones16[:], ones_f[:])
        else:
            ones16 = const.tile([1, P], f16)
            nc.gpsimd.memset(ones16[:], 1.0)

        # ---- w_outT[e, d] -> DRAM scratch [OC, P, EC, OF] f16 ----
        # Layout chosen so the out-phase readback is one contiguous
        # EC*OF*2-byte run per partition (single-descriptor DMA).
        w_dram = dram.tile([OC, P, EC, OF], f16)
        with tc.tile_pool(name="wnat", bufs=2) as wnat, \
             tc.tile_pool(name="wsb", bufs=2) as wsb:
            for dt_ in range(DC):
                dc2, od0 = (dt_ * P) // OF, (dt_ * P) % OF
                wtile = wnat.tile([P, E], f32, tag="wnat")
                nc.sync.dma_start(wtile[:], w_out[dt_ * P:(dt_ + 1) * P, :])
                for j0 in range(0, EC, 4):
                    nj = min(4, EC - j0)
                    ps = ps_tp.tile([P, 4 * P], f32, tag="tp")
                    for a in range(nj):
                        nc.tensor.transpose(
                            ps[:, a * P:(a + 1) * P],
                            wtile[:, (j0 + a) * P:(j0 + a + 1) * P],
                            ident32,
                        )
                    wsl = wsb.tile([P, 4 * P], f16, tag="wsb")
                    nc.vector.tensor_copy(wsl[:, :nj * P], ps[:, :nj * P])
                    nc.sync.dma_start(
                        w_dram[dc2, :, j0:j0 + nj, od0:od0 + P],
                        wsl[:, :nj * P].rearrange("p (j d) -> p j d", j=nj),
                    )

        # ---- persistent per-batch pools ----
        perb = ctx.enter_context(tc.tile_pool(name="perb", bufs=1))
        mrow = ctx.enter_context(tc.tile_pool(name="mrow", bufs=1))
        nat = ctx.enter_context(tc.tile_pool(name="nat", bufs=3))
        chh = ctx.enter_context(tc.tile_pool(name="chh", bufs=2))
        ctp = ctx.enter_context(tc.tile_pool(name="ctp", bufs=2))
        stp = ctx.enter_context(tc.tile_pool(name="stp", bufs=2))
        ptp = ctx.enter_context(tc.tile_pool(name="ptp", bufs=KG + 2))
        smal = ctx.enter_context(tc.tile_pool(name="smal", bufs=4))
        stg = ctx.enter_context(tc.tile_pool(name="stg", bufs=2))
        mixp = ctx.enter_context(tc.tile_pool(name="mixp", bufs=2))
        strm = ctx.enter_context(tc.tile_pool(name="strm", bufs=2))
        cstr = ctx.enter_context(tc.tile_pool(name="cstr", bufs=3))
        outp = ctx.enter_context(tc.tile_pool(name="outp", bufs=3))
        qstg = ctx.enter_context(tc.tile_pool(name="qstg", bufs=2)) if use_f32r else None

        # pT spill: [b, stripe, partition, kt, qf] -> stripe readback is one
        # contiguous KT*QF*2-byte run per partition.
        pT_dram = dram.tile([B_pc, QC, P, KT, QF], f16)

        for b in range(B_pc):
            # ctx(f16) spill: [j, partition(k), kt, pd] -> mix readback of
            # [j] is one contiguous KT*P*2-byte run per partition.
            ctx_dram = dram.tile([DC, P, KT, P], f16, tag="ctxd", name="ctxd")

            # ---- mask bias row: (m - 1) * 60000, f16 (folded into the scores
            # matmul as a rank-1 ones x mb pass; exp(-60000+x) == 0 in f32) ----
            mrow_i = mrow.tile([1, Q], i32, tag="mrow_i")
            nc.sync.dma_start(mrow_i[:], amask[b, None, :])
            mrow_f = mrow.tile([1, Q], f32, tag="mrow_f")
            nc.vector.tensor_copy(mrow_f[:], mrow_i[:])
            mb16 = mrow.tile([1, Q], f32r if use_f32r else f16, tag="mb16")
            nc.vector.tensor_scalar(
                mb16[:], mrow_f[:], 60000.0, -60000.0, op0=Alu.mult, op1=Alu.add
            )

            # ---- qT (hi/lo fp16, or f32 for the f32r mode) ----
            # layout [P, DC, Q], partition = d in chunk
            qT_hi = perb.tile([P, DC, Q], f32r if use_f32r else f16, tag="qt_hi")
            qT_lo = (
                perb.tile([P, DC, Q], f16, tag="qt_lo", name="qt_lo")
                if score_passes == 3
                else None
            )
            for t in range(QT):
                qtile = nat.tile([P, D], f32, tag="nat")
                nc.sync.dma_start(qtile[:], query[b, t * P:(t + 1) * P, :])
                for j0 in range(0, DC, 4):
                    nj = min(4, DC - j0)
                    ps = ps_tp.tile([P, 4 * P], f32, tag="tp")
                    for a in range(nj):
                        nc.tensor.transpose(
                            ps[:, a * P:(a + 1) * P],
                            qtile[:, (j0 + a) * P:(j0 + a + 1) * P],
                            ident32,
                        )
                    psv = ps[:, :nj * P].rearrange("p (a c) -> p a c", a=nj)
                    dst = qT_hi[:, j0:j0 + nj, t * P:(t + 1) * P]
                    if t % 2 == 0 or use_f32r:
                        nc.vector.tensor_copy(dst, psv)
                    else:
                        nc.scalar.copy(dst, psv)
                    if qT_lo is not None:
                        nc.vector.tensor_tensor(
                            qT_lo[:, j0:j0 + nj, t * P:(t + 1) * P],
                            psv,
                            dst,
                            Alu.subtract,
                        )

            # ---- k loop: scores -> softmax -> pT (spilled to DRAM) ----
            pT_tiles = [None] * KT

            def emit_attw_group(g, b=b):
                # transpose pT tiles [kG..] back to [q, k] layout and DMA out
                for qb in range(QTA):
                    pp = ps_pt.tile([P, KG * P], f16, tag="pt")
                    for a in range(KG):
                        nc.tensor.transpose(
                            pp[:, a * P:(a + 1) * P],
                            pT_tiles[g * KG + a][:, qb * P:(qb + 1) * P],
                            ident16,
                        )
                    st = stg.tile([P, KG * P], f32, tag="stg")
                    if qb % 2 == 0:
                        nc.vector.tensor_copy(st[:], pp[:])
                    else:
                        nc.scalar.copy(st[:], pp[:])
                    nc.sync.dma_start(
                        attw[b, qb * P:(qb + 1) * P, g * KG * P:(g + 1) * KG * P],
                        st[:],
                    )

            for kt in range(KT):
                ctile = nat.tile([P, D], f32, tag="nat")
                nc.sync.dma_start(ctile[:], context[b, kt * P:(kt + 1) * P, :])
                chf = chh.tile([P, D], f16, tag="chh")
                nc.gpsimd.tensor_copy(chf[:], ctile[:])
                nc.sync.dma_start(
                    ctx_dram[:, :, kt, :].rearrange("j pk pd -> pk j pd"),
                    chf[:].rearrange("pk (j pd) -> pk j pd", j=DC),
                )

                cT_hi = ctp.tile([P, DC, P], f32r if use_f32r else f16, tag="ct_hi")
                cT_lo = (
                    ctp.tile([P, DC, P], f16, tag="ct_lo", name="ct_lo")
                    if score_passes == 3
                    else None
                )
                for j0 in range(0, DC, 4):
                    nj = min(4, DC - j0)
                    ps = ps_tp.tile([P, 4 * P], f32, tag="tp")
                    for a in range(nj):
                        nc.tensor.transpose(
                            ps[:, a * P:(a + 1) * P],
                            ctile[:, (j0 + a) * P:(j0 + a + 1) * P],
                            ident32,
                        )
                    psv = ps[:, :nj * P].rearrange("p (a c) -> p a c", a=nj)
                    dst = cT_hi[:, j0:j0 + nj, :]
                    if kt % 2 == 0 or use_f32r:
                        nc.vector.tensor_copy(dst, psv)
                    else:
                        nc.scalar.copy(dst, psv)
                    if cT_lo is not None:
                        nc.vector.tensor_tensor(
                            cT_lo[:, j0:j0 + nj, :],
                            psv,
                            dst,
                            Alu.subtract,
                        )

                # scores for this k tile, all active q chunks -> sT (masked, f32)
                sT = stp.tile([P, qa], f32, tag="sT")
                if score_passes == 3:
                    passes = [(cT_hi, qT_hi), (cT_hi, qT_lo), (cT_lo, qT_hi)]
                else:
                    passes = [(cT_hi, qT_hi)]
                # All QC chunk accumulators live at once so each lhsT weight
                # tile is loaded once and reused across the QC matmuls
                # (f32r weight loads are slow; fp16 FWL also benefits).
                nmm = len(passes) * DC
                pmms = [
                    ps_mm.tile([P, QF], f32, tag="mm", name=f"mm{qc}")
                    for qc in range(QC)
                ]
                for qc in range(QC):
                    # rank-1 mask pass: psum += ones[k] x maskbias[q]
                    nc.tensor.matmul(
                        pmms[qc][:],
                        lhsT=ones16[:],
                        rhs=mb16[:, qc * QF:(qc + 1) * QF],
                        start=True,
                        stop=False,
                    )
                i = 0
                for lhsP, rhsP in passes:
                    for j in range(DC):
                        for qc in range(QC):
                            nc.tensor.matmul(
                                pmms[qc][:],
                                lhsT=lhsP[:, j, :],
                                rhs=rhsP[:, j, qc * QF:(qc + 1) * QF],
                                start=False,
                                stop=(i == nmm - 1),
                            )
                        i += 1
                for qc in range(QC):
                    if qc % 2 == 0:
                        nc.scalar.copy(sT[:, qc * QF:(qc + 1) * QF], pmms[qc][:])
                    else:
                        nc.vector.tensor_copy(sT[:, qc * QF:(qc + 1) * QF], pmms[qc][:])

                # softmax over q (free axis)
                negmx = smal.tile([P, 1], f32, tag="negmx")
                nc.vector.reduce_max(
                    negmx[:], sT[:], axis=mybir.AxisListType.X, negate=True
                )
                pT = ptp.tile([P, qa], f16, tag="pT")
                denom = smal.tile([P, 1], f32, tag="denom")
                nc.scalar.activation(
                    pT[:], sT[:], Act.Exp, bias=negmx[:], scale=1.0, accum_out=denom[:]
                )
                recip = smal.tile([P, 1], f32, tag="recip")
                nc.vector.reciprocal(recip[:], denom[:])
                nc.vector.tensor_scalar_mul(pT[:], pT[:], recip[:])
                pT_tiles[kt] = pT
                nc.gpsimd.dma_start(
                    pT_dram[b, :, :, kt, :].rearrange("s p f -> p s f"),
                    pT[:].rearrange("p (s f) -> p s f", s=QC),
                )

                # emit attw transposes for the previous completed group (keeps
                # PE fed: by now that group's softmax has long finished)
                if kt % KG == 0 and kt > 0:
                    emit_attw_group(kt // KG - 1)
            emit_attw_group(NG - 1)

            # ---- mix + out, per q stripe ----
            for s in range(QC):
                pst = strm.tile([P, KT, QF], f16, tag="strm", name="pstrip")
                nc.sync.dma_start(pst[:], pT_dram[b, s])
                mixT = mixp.tile([P, DC, QF], f16, tag="mixT")
                for j in range(DC):
                    cs = cstr.tile([P, KT, P], f16, tag="cstr")
                    nc.sync.dma_start(cs[:], ctx_dram[j])
                    pmm = ps_mm.tile([P, QF], f32, tag="mm")
                    for kt in range(KT):
                        nc.tensor.matmul(
                            pmm[:],
                            lhsT=cs[:, kt, :],
                            rhs=pst[:, kt, :],
                            start=(kt == 0),
                            stop=(kt == KT - 1),
                        )
                    nc.vector.tensor_copy(mixT[:, j, :], pmm[:])

                for dc2 in range(OC):
                    ws = strm.tile([P, EC, OF], f16, tag="strm", name="wstrip")
                    nc.sync.dma_start(ws[:], w_dram[dc2])
                    for qs in range(QF // P):
                        qv = qT_hi[:, :, s * QF + qs * P:s * QF + (qs + 1) * P]
                        if use_f32r:
                            # stage the fp32 qT slice as f16 for the mixed out
                            # matmul (walrus requires matching operand dtypes)
                            qst = qstg.tile([P, DC, P], f16, tag="qstg")
                            nc.vector.tensor_copy(qst[:], qv)
                            qv = qst
                        pmm = ps_mm.tile([P, OF], f32, tag="mm")
                        for j in range(DC):
                            nc.tensor.matmul(
                                pmm[:],
                                lhsT=mixT[:, j, qs * P:(qs + 1) * P],
                                rhs=ws[:, j, :],
                                start=(j == 0),
                                stop=False,
                            )
                        for j in range(DC):
                            nc.tensor.matmul(
                                pmm[:],
                                lhsT=qv[:, j, :],
                                rhs=ws[:, DC + j, :],
                                start=False,
                                stop=(j == DC - 1),
                            )
                        osb = outp.tile([P, OF], f32, tag="osb")
                        nc.scalar.activation(osb[:], pmm[:], Act.Tanh)
                        r0 = s * QF + qs * P
                        nc.gpsimd.dma_start(
                            out[b, r0:r0 + P, dc2 * OF:(dc2 + 1) * OF], osb[:]
                        )

            # ---- masked tail rows [qa, Q): attention weights are exactly
            # zero there, so only the query half of w_out contributes ----
            for dc2 in range(OC if QTA < QT else 0):
                wst = strm.tile([P, EC, OF], f16, tag="strm", name="wtail")
                nc.sync.dma_start(wst[:], w_dram[dc2])
                for blk in range(QTA, QT):
                    qv = qT_hi[:, :, blk * P:(blk + 1) * P]
                    if use_f32r:
                        qst = qstg.tile([P, DC, P], f16, tag="qstg")
                        nc.vector.tensor_copy(qst[:], qv)
                        qv = qst
                    pmm = ps_mm.tile([P, OF], f32, tag="mm")
                    for j in range(DC):
                        nc.tensor.matmul(
                            pmm[:],
                            lhsT=qv[:, j, :],
                            rhs=wst[:, DC + j, :],
                            start=(j == 0),
                            stop=(j == DC - 1),
                        )
                    osb = outp.tile([P, OF], f32, tag="osb")
                    nc.scalar.activation(osb[:], pmm[:], Act.Tanh)
                    nc.gpsimd.dma_start(
                        out[b, blk * P:(blk + 1) * P, dc2 * OF:(dc2 + 1) * OF],
                        osb[:],
                    )

    nc.compile()
    return nc


def _get_nc(key):
    if key not in _NC_CACHE:
        _NC_CACHE[key] = build_nc(*key)
    return _NC_CACHE[key]


def run_sharded(inputs, trace=False, score_passes=2, compact=True, **spmd_kwargs):
    from concourse.bass_utils import run_bass_kernel_spmd

    query = np.ascontiguousarray(np.asarray(inputs["query"], dtype=np.float32))
    context = np.ascontiguousarray(np.asarray(inputs["context"], dtype=np.float32))
    mask = np.ascontiguousarray(np.asarray(inputs["attention_mask"], dtype=np.int32))
    w_out = np.ascontiguousarray(np.asarray(inputs["w_out"], dtype=np.float32))

    B, Q, D = query.shape
    K = context.shape[1]
    n_cores = 8
    B_pc = B // n_cores

    # Permute each batch's rows so the active (mask==1) rows come first; the
    # kernel then only runs scores/softmax/attw/mix on the first `qa` rows.
    # Pure row reordering: per-row results are numerically identical.
    qa = Q
    perms = None
    if compact:
        n_max = int(mask.sum(axis=1).max())
        qa = min(Q, max(384, -(-n_max // 384) * 384))
        if qa < Q:
            perms = np.argsort(1 - mask, axis=1, kind="stable")
            query = np.take_along_axis(query, perms[:, :, None], axis=1)
            mask = np.take_along_axis(mask, perms, axis=1)
        else:
            qa = Q

    nc = _get_nc((B_pc, Q, K, D, score_passes, qa))

    in_maps = [
        {
            "query": query[i * B_pc:(i + 1) * B_pc],
            "context": context[i * B_pc:(i + 1) * B_pc],
            "attention_mask": mask[i * B_pc:(i + 1) * B_pc],
            "w_out": w_out,
        }
        for i in range(n_cores)
    ]
    res = run_bass_kernel_spmd(
        nc, in_maps, core_ids=list(range(n_cores)), trace=trace, **spmd_kwargs
    )
    outs = res.results
    out = np.concatenate([r["out"] for r in outs], axis=0)
    attw = np.concatenate([r["attw"] for r in outs], axis=0)

    if perms is not None:
        out_f = np.empty_like(out)
        attw_f = np.zeros_like(attw)
        for b in range(B):
            out_f[b, perms[b]] = out[b]
            attw_f[b, perms[b, :qa]] = attw[b, :qa]
        out, attw = out_f, attw_f
    return out, attw, res


def kernel(**inputs):
    out, attw, _ = run_sharded(inputs)
    return out, attw
